# revision 42
# baseline (speedup 1.0000x reference)
"""MiniGPT (L=8, E=1024, H=16, T=1024, B=4, V=32000) on 8 TRN2 NeuronCores.

Sharding: data-parallel over (batch, sequence-half) -> 8 shards of 512 tokens.
All weights replicated per core. Per layer, the two cores sharing a batch
exchange K/V via pair AllGathers. Causal masking is data-driven (per-core
mask tables) so the SPMD program is uniform across cores.

v2 changes vs baseline:
 - all big matmuls run with bf16 operands on both sides (fp32 moving operand
   streams at half rate); residual stream and LN stats stay f32.
 - weight DMAs batched: one [128, 8, 512] block load per 512-col chunk,
   fc2 weights host-relaid so each e-tile's lhsT is one contiguous DMA.
 - score matmuls for head pairs issued back-to-back with lhsT base
   partitions 0/64 so the PE overlaps them via row-group tiling.
 - attention softmax reciprocal + LN rsqrt on the scalar engine (the DVE
   reciprocal instruction measured 3.35us each).
 - LN gain/bias folded into the following weights on the host; K/Q biases
   applied at psum evacuation, V bias deferred through proj into the
   residual add.
 - K/V gathered-tile reloads ride the Activation-engine DMA queue so they
   don't head-of-line block weight streaming on the sync queue.
"""
import sys

sys.path.insert(0, "/opt/trn_rl_repo")

import numpy as np
import ml_dtypes

import concourse.bass as bass
import concourse.bacc as bacc
import concourse.mybir as mybir
import concourse.tile as tile
from concourse.bass_utils import run_bass_kernel_spmd

V, E, H, L, T, B = 32000, 1024, 16, 8, 1024, 4
D = E // H              # 64
F = 4 * E               # 4096
EPS = 1e-5
TOK = 512               # tokens per core
NCORES = 8
ET = E // 128            # 8 feature tiles
FT = F // 128            # 32 mlp-hidden tiles
SCALE = 1.0 / np.sqrt(D)

F32 = mybir.dt.float32
F32R = mybir.dt.float32r
BF16 = mybir.dt.bfloat16
AF = mybir.ActivationFunctionType
ALU = mybir.AluOpType

PAIRS = [[0, 1], [2, 3], [4, 5], [6, 7]]
# LM head: 31 blocks of 1024 cols + 1 block of 256
HEAD_BLOCKS = [(i * 1024, min(1024, V - i * 1024)) for i in range(32)]

_CACHED = {}


def _build_nc():
    nc = bacc.Bacc("TRN2", target_bir_lowering=False, debug=False,
                   num_devices=NCORES)

    def P(name, shape, dt, out=False):
        return nc.declare_dram_parameter(name, list(shape), dt, isOutput=out)

    x0T = P("x0T", [E, TOK], F32R)                 # per-core residual seed
    wqkvT = P("wqkvT", [L, E, 3 * E], BF16)        # cols: [K | V | Q], ln1_g folded
    wprojT = P("wprojT", [L, E, E], BF16)
    w1T = P("w1T", [L, E, F], BF16)                # ln2_g folded
    w2r = P("w2r", [L, ET, 128, FT * 128], BF16)   # fc2 lhsT blocks per e-tile
    b1c = P("b1c", [L, 128, FT], F32)              # fc1 bias (+ fc1_w@ln2_b)
    b2c = P("b2c", [L, 128, ET], F32)              # fc2 bias as columns
    prb = P("prb", [L, 128, ET], F32)              # proj_w @ (v-bias) columns
    kqb = P("kqb", [L, 128, 2 * ET], F32)          # K-bias cols | Q-bias cols
    lnf = P("lnf", [2, 128, ET], F32)              # lnf_g, lnf_b
    headT = P("headT", [E, V], BF16)
    masks = P("masks", [8, 128, TOK], BF16)        # per-core causal masks
    ones_p = P("ones_p", [128, 16], F32R)          # all-ones helper (f32)
    ones_bf = P("ones_bf", [128, 128], BF16)       # all-ones helper (bf16)
    logits = P("logits", [TOK, V], F32, out=True)

    with tile.TileContext(nc) as tc:
        with (
            tc.tile_pool(name="persist", bufs=1) as persist,
            tc.tile_pool(name="acts", bufs=8) as acts,         # h1/YT/h2 [128,512] bf16
            tc.tile_pool(name="qt", bufs=8) as qtp,            # QT [128,512] bf16
            tc.tile_pool(name="ut", bufs=FT) as utp,           # [128,512] bf16
            tc.tile_pool(name="wq", bufs=3) as wq,             # [128,8,512] bf16 blocks
            tc.tile_pool(name="w2", bufs=2) as w2p,            # [128,FT,128] bf16
            tc.tile_pool(name="stg", bufs=3) as stg,           # [128,512] staging
            tc.tile_pool(name="lo", bufs=5) as lop,            # [128,1024] f32 head out
            tc.tile_pool(name="pp", bufs=4) as pp,             # probs bf16 / ysb
            tc.tile_pool(name="vec", bufs=2) as vec,
            tc.tile_pool(name="sm", bufs=5) as sm,             # [1,512] stats
            tc.tile_pool(name="ps", bufs=2, space="PSUM") as ps,
            tc.tile_pool(name="pse", bufs=2, space="PSUM") as pse,
            tc.tile_pool(name="psy", bufs=2, space="PSUM") as psy,
            tc.tile_pool(name="dram", bufs=2, space="DRAM") as dram,
        ):
            # ---- persistent tiles ----
            xT = [persist.tile([128, TOK], F32R, tag=f"xT{e}", name=f"xT{e}")
                  for e in range(ET)]
            KTa = persist.tile([128, 4, 2 * TOK], BF16, tag="KTa", name="KTa")
            KTb = persist.tile([128, 4, 2 * TOK], BF16, tag="KTb", name="KTb")
            VS = persist.tile([128, 8, H * 65], BF16, tag="VS", name="VS")
            MK = [persist.tile([128, 2 * TOK], BF16, tag=f"MK{k}", name=f"MK{k}")
                  for k in range(8)]
            ones_col = persist.tile([128, 1], F32R, tag="ones_col")
            ones_row = persist.tile([1, 128], F32R, tag="ones_row")
            eps_t = persist.tile([1, 1], F32, tag="eps")
            nc.sync.dma_start(out=ones_col[:], in_=ones_p[:, 0:1])
            nc.sync.dma_start(out=ones_row[:],
                              in_=ones_p.rearrange("a b -> (a b)")[0:128])
            nc.vector.memset(eps_t[:], EPS)
            # ones column (index 64) of every VS head slot, set once;
            # the per-layer V loads only touch indices 0..63.
            nc.vector.memset(
                VS[:].rearrange("p t (h e) -> p (t h) e", e=65)[:, :, 64:65], 1.0)

            for e in range(ET):
                nc.sync.dma_start(out=xT[e][:], in_=x0T[e * 128:(e + 1) * 128, :])
            for k in range(8):
                nc.sync.dma_start(out=MK[k][:, 0:TOK], in_=masks[k])
                nc.sync.dma_start(out=MK[k][:, TOK:2 * TOK], in_=masks[k])

            def layernorm(src, affine=None):
                """src: ET [128,TOK] f32r tiles. Returns ET bf16 tiles (acts
                pool). If affine=(g_ap, b_ap) apply per-feature gain/bias
                ([128, ET] column APs); otherwise gain/bias are pre-folded
                into the consuming weights."""
                psum = ps.tile([1, TOK], F32, tag="bank")
                psq = ps.tile([1, TOK], F32, tag="bank")
                for e in range(ET):
                    sq = stg.tile([128, TOK], F32R, tag="stg")
                    nc.scalar.activation(sq[:], src[e][:], AF.Square)
                    nc.tensor.matmul(psum[:], lhsT=ones_col[:], rhs=src[e][:],
                                     start=(e == 0), stop=(e == ET - 1))
                    nc.tensor.matmul(psq[:], lhsT=ones_col[:], rhs=sq[:],
                                     start=(e == 0), stop=(e == ET - 1))
                mu = sm.tile([1, TOK], F32, tag="sm")
                var = sm.tile([1, TOK], F32, tag="sm")
                mu2 = sm.tile([1, TOK], F32, tag="sm")
                rstd = sm.tile([1, TOK], F32R, tag="sm")
                nmr = sm.tile([1, TOK], F32R, tag="sm")
                nc.scalar.activation(mu[:], psum[:], AF.Identity, scale=1.0 / E)
                nc.scalar.activation(var[:], psq[:], AF.Identity, scale=1.0 / E)
                nc.vector.tensor_mul(mu2[:], mu[:], mu[:])
                nc.vector.tensor_sub(var[:], var[:], mu2[:])
                # rstd = exp(-0.5 * ln(var + eps)) = 1/sqrt(var + eps)
                nc.scalar.activation(var[:], var[:], AF.Ln, bias=eps_t[:])
                with nc.allow_low_precision(reason="f32r rounding for matmul rhs"):
                    nc.scalar.activation(rstd[:], var[:], AF.Exp, scale=-0.5)
                nc.vector.tensor_mul(nmr[:], mu[:], rstd[:])
                nc.vector.tensor_scalar_mul(nmr[:], nmr[:], -1.0)
                A = ps.tile([128, TOK], F32, tag="bank")
                C = ps.tile([128, TOK], F32, tag="bank")
                nc.tensor.matmul(A[:], lhsT=ones_row[:], rhs=rstd[:],
                                 start=True, stop=True)
                nc.tensor.matmul(C[:], lhsT=ones_row[:], rhs=nmr[:],
                                 start=True, stop=True)
                out = []
                for e in range(ET):
                    t = stg.tile([128, TOK], F32, tag="stg")
                    nc.vector.tensor_mul(t[:], src[e][:], A[:])
                    h = acts.tile([128, TOK], BF16, tag="acts")
                    if affine is None:
                        nc.vector.tensor_add(h[:], t[:], C[:])
                    else:
                        t2 = stg.tile([128, TOK], F32, tag="stg")
                        nc.vector.tensor_add(t2[:], t[:], C[:])
                        g_ap, b_ap = affine
                        nc.scalar.activation(h[:], t2[:], AF.Identity,
                                             scale=g_ap[:, e:e + 1],
                                             bias=b_ap[:, e:e + 1])
                    out.append(h)
                return out

            def load_w8(src2d, c0, w):
                """One DMA: 8 [128, w] k-tiles covering cols c0:c0+w."""
                wt = wq.tile([128, 8, 512], BF16, tag="wq", name="wt")
                nc.sync.dma_start(
                    out=wt[:, :, 0:w],
                    in_=src2d.rearrange("(k p) c -> p k c", p=128)[:, :, c0:c0 + w])
                return wt

            for l in range(L):
                # per-layer vectors
                b1t = vec.tile([128, FT], F32, tag="b1t")
                nc.sync.dma_start(out=b1t[:], in_=b1c[l])
                b2t = vec.tile([128, ET], F32, tag="b2t")
                nc.sync.dma_start(out=b2t[:], in_=b2c[l])
                prt = vec.tile([128, ET], F32, tag="prt")
                nc.sync.dma_start(out=prt[:], in_=prb[l])
                kqt = vec.tile([128, 2 * ET], F32, tag="kqt")
                nc.sync.dma_start(out=kqt[:], in_=kqb[l])

                # ---- LN1 ----
                h1 = layernorm(xT)

                # ---- qkv: K rows -> gather, V -> gather, then Q ----
                stage_k = dram.tile([E, TOK], BF16, tag="stgk")
                full_k1 = dram.tile([2, 512, TOK], BF16, tag="fullk1")
                full_k2 = dram.tile([2, 512, TOK], BF16, tag="fullk2")
                stage_v = dram.tile([TOK, E], BF16, tag="stgv")
                full_v = dram.tile([2, TOK, E], BF16, tag="fullv")

                # K gathered in two halves so the first half's gather overlaps
                # the second half's compute and attention pairs 0-3 unblock
                # as early as possible.
                for cb in range(2):          # K rows (wqkvT cols 0..1023)
                    wt = load_w8(wqkvT[l], cb * 512, 512)
                    for r in range(4):
                        row = cb * 4 + r
                        pk = ps.tile([128, TOK], F32, tag="bank")
                        for k in range(ET):
                            nc.tensor.matmul(pk[:],
                                             lhsT=wt[:, k, r * 128:(r + 1) * 128],
                                             rhs=h1[k][:], start=(k == 0),
                                             stop=(k == ET - 1))
                        ksb = stg.tile([128, TOK], BF16, tag="stg")
                        nc.vector.tensor_scalar_add(ksb[:], pk[:],
                                                    kqt[:, row:row + 1])
                        nc.sync.dma_start(
                            out=stage_k[row * 128:(row + 1) * 128, :], in_=ksb[:])
                    nc.gpsimd.collective_compute(
                        "AllGather", ALU.bypass, replica_groups=PAIRS,
                        ins=[stage_k[cb * 512:(cb + 1) * 512, :]],
                        outs=[(full_k1 if cb == 0 else full_k2)[:]])

                for cb in range(2):          # V (wqkvT cols 1024..2047)
                    wt = load_w8(wqkvT[l], 1024 + cb * 512, 512)
                    for t in range(4):
                        pv = ps.tile([128, 512], F32, tag="bank")
                        for k in range(ET):
                            nc.tensor.matmul(pv[:],
                                             lhsT=h1[k][:, t * 128:(t + 1) * 128],
                                             rhs=wt[:, k, :], start=(k == 0),
                                             stop=(k == ET - 1))
                        vsb = stg.tile([128, 512], BF16, tag="stg")
                        nc.vector.tensor_copy(out=vsb[:], in_=pv[:])
                        nc.sync.dma_start(
                            out=stage_v[t * 128:(t + 1) * 128,
                                        cb * 512:(cb + 1) * 512], in_=vsb[:])
                nc.gpsimd.collective_compute(
                    "AllGather", ALU.bypass, replica_groups=PAIRS,
                    ins=[stage_v[:]], outs=[full_v[:]])

                QT = []
                for cb in range(2):          # Q rows (wqkvT cols 2048..3071)
                    wt = load_w8(wqkvT[l], 2048 + cb * 512, 512)
                    for r in range(4):
                        row = cb * 4 + r
                        pq = ps.tile([128, TOK], F32, tag="bank")
                        for k in range(ET):
                            nc.tensor.matmul(pq[:],
                                             lhsT=wt[:, k, r * 128:(r + 1) * 128],
                                             rhs=h1[k][:], start=(k == 0),
                                             stop=(k == ET - 1))
                        q = qtp.tile([128, TOK], BF16, tag="qt")
                        nc.vector.tensor_scalar_add(q[:], pq[:],
                                                    kqt[:, 8 + row:9 + row])
                        QT.append(q)

                # load gathered K/V (global key order: rank0 | rank1) on the
                # Activation-engine DMA queue (avoids blocking weight loads).
                # Both KT loads go first so the score matmuls can start while
                # the V gather is still in flight.
                for rk in range(2):
                    nc.scalar.dma_start(
                        out=KTa[:, :, rk * TOK:(rk + 1) * TOK],
                        in_=full_k1[rk].rearrange("(r p) t -> p r t", p=128))
                for rk in range(2):
                    nc.scalar.dma_start(
                        out=KTb[:, :, rk * TOK:(rk + 1) * TOK],
                        in_=full_k2[rk].rearrange("(r p) t -> p r t", p=128))
                for rk in range(2):
                    for t in range(4):
                        nc.scalar.dma_start(
                            out=VS[:, rk * 4 + t, :].rearrange(
                                "p (h e) -> p h e", e=65)[:, :, 0:64],
                            in_=full_v[rk, t * 128:(t + 1) * 128, :].rearrange(
                                "p (h d) -> p h d", d=64))

                # ---- attention (heads 2r / 2r+1 share KT row r; their score
                # matmuls use lhsT base partitions 0/64 -> PE row groups) ----
                YT = [acts.tile([128, TOK], BF16, tag="acts", name="yt")
                      for _ in range(ET)]
                for r in range(ET):
                    h0, h1h = 2 * r, 2 * r + 1
                    KTh = KTa if r < 4 else KTb
                    rr = r if r < 4 else r - 4
                    py0 = psy.tile([65, TOK], F32, tag="ybank")
                    py1 = psy.tile([65, TOK], F32, tag="ybank")
                    for kt_idx in range(8):
                        sc2 = pse.tile([128, 2 * TOK], F32, tag="pse")
                        nc.tensor.matmul(
                            sc2[:, 0:TOK],
                            lhsT=KTh[0:64, rr, kt_idx * 128:(kt_idx + 1) * 128],
                            rhs=QT[r][0:64, :], start=True, stop=True)
                        nc.tensor.matmul(
                            sc2[:, TOK:2 * TOK],
                            lhsT=KTh[64:128, rr, kt_idx * 128:(kt_idx + 1) * 128],
                            rhs=QT[r][64:128, :], start=True, stop=True)
                        pr2 = pp.tile([128, 2 * TOK], BF16, tag="pp")
                        nc.scalar.activation(pr2[:], sc2[:], AF.Exp,
                                             scale=float(SCALE))
                        nc.vector.tensor_mul(pr2[:], pr2[:], MK[kt_idx][:])
                        nc.tensor.matmul(
                            py0[:], lhsT=VS[:, kt_idx, h0 * 65:(h0 + 1) * 65],
                            rhs=pr2[:, 0:TOK], start=(kt_idx == 0),
                            stop=(kt_idx == 7))
                        nc.tensor.matmul(
                            py1[:], lhsT=VS[:, kt_idx, h1h * 65:(h1h + 1) * 65],
                            rhs=pr2[:, TOK:2 * TOK], start=(kt_idx == 0),
                            stop=(kt_idx == 7))
                    for po, py in ((0, py0), (64, py1)):
                        ld = sm.tile([1, TOK], F32, tag="sm")
                        rec = sm.tile([1, TOK], F32R, tag="sm")
                        # rec = exp(-ln(denom)) = 1/denom
                        nc.scalar.activation(ld[:], py[64:65, :], AF.Ln)
                        with nc.allow_low_precision(reason="matmul rhs"):
                            nc.scalar.activation(rec[:], ld[:], AF.Exp,
                                                 scale=-1.0)
                        pb = ps.tile([64, TOK], F32, tag="bank")
                        nc.tensor.matmul(pb[:], lhsT=ones_row[:, 0:64],
                                         rhs=rec[:], start=True, stop=True)
                        ysb = pp.tile([128, TOK], F32, tag="ysb", name="ysb")
                        nc.vector.tensor_copy(out=ysb[0:64, :], in_=py[0:64, :])
                        nc.vector.tensor_mul(YT[r][po:po + 64, :],
                                             ysb[0:64, :], pb[:])

                # ---- proj + residual (+ deferred V-bias via prt) ----
                for cb in range(2):
                    wt = load_w8(wprojT[l], cb * 512, 512)
                    for r in range(4):
                        e = cb * 4 + r
                        pe = ps.tile([128, TOK], F32, tag="bank")
                        for k in range(ET):
                            nc.tensor.matmul(pe[:],
                                             lhsT=wt[:, k, r * 128:(r + 1) * 128],
                                             rhs=YT[k][:], start=(k == 0),
                                             stop=(k == ET - 1))
                        nc.vector.scalar_tensor_tensor(
                            out=xT[e][:], in0=pe[:], scalar=prt[:, e:e + 1],
                            in1=xT[e][:], op0=ALU.add, op1=ALU.add)

                # ---- LN2 ----
                h2 = layernorm(xT)

                # ---- fc1 + gelu ----
                uT = []
                for cb in range(8):          # 8 chunks of 512 hidden cols
                    wt = load_w8(w1T[l], cb * 512, 512)
                    for r in range(4):
                        uc = cb * 4 + r
                        pu = ps.tile([128, TOK], F32, tag="bank")
                        for k in range(ET):
                            nc.tensor.matmul(pu[:],
                                             lhsT=wt[:, k, r * 128:(r + 1) * 128],
                                             rhs=h2[k][:], start=(k == 0),
                                             stop=(k == ET - 1))
                        u = utp.tile([128, TOK], BF16, tag="ut")
                        nc.scalar.activation(u[:], pu[:], AF.Gelu,
                                             bias=b1t[:, uc:uc + 1])
                        uT.append(u)

                # ---- fc2 + bias + residual ----
                for e in range(ET):
                    w2t = w2p.tile([128, FT, 128], BF16, tag="w2", name="w2t")
                    nc.sync.dma_start(
                        out=w2t[:],
                        in_=w2r[l, e].rearrange("p (q m) -> p q m", m=128))
                    pe = ps.tile([128, TOK], F32, tag="bank")
                    for uc in range(FT):
                        nc.tensor.matmul(
                            pe[:], lhsT=w2t[:, uc, :], rhs=uT[uc][:],
                            start=(uc == 0), stop=(uc == FT - 1))
                    nc.vector.scalar_tensor_tensor(
                        out=xT[e][:], in0=pe[:], scalar=b2t[:, e:e + 1],
                        in1=xT[e][:], op0=ALU.add, op1=ALU.add)

            # ---- final LN (with affine) + head ----
            lnft = vec.tile([128, 2 * ET], F32, tag="lnft")
            nc.sync.dma_start(out=lnft[:], in_=lnf.rearrange("a p b -> p a b"))
            xf = layernorm(xT, affine=(lnft[:, 0:ET], lnft[:, ET:2 * ET]))

            headv = headT.rearrange("(k p) v -> p k v", p=128)
            for bi, (voff, vlen) in enumerate(HEAD_BLOCKS):
                lo = [lop.tile([128, 1024], F32, tag="lo", name=f"lo{t}")
                      for t in range(4)]
                subs = [(s * 512, min(512, vlen - s * 512))
                        for s in range((vlen + 511) // 512)]
                for si, (soff, slen) in enumerate(subs):
                    wh = wq.tile([128, 8, 512], BF16, tag="wq", name="wh")
                    nc.sync.dma_start(
                        out=wh[:, :, 0:slen],
                        in_=headv[:, :, voff + soff:voff + soff + slen])
                    for t in range(4):
                        pl = ps.tile([128, 512], F32, tag="bank")
                        for k in range(ET):
                            nc.tensor.matmul(
                                pl[:, 0:slen],
                                lhsT=xf[k][:, t * 128:(t + 1) * 128],
                                rhs=wh[:, k, 0:slen],
                                start=(k == 0), stop=(k == ET - 1))
                        if t % 2 == 0:
                            nc.vector.tensor_copy(out=lo[t][:, soff:soff + slen],
                                                  in_=pl[:, 0:slen])
                        else:
                            nc.scalar.activation(lo[t][:, soff:soff + slen],
                                                 pl[:, 0:slen], AF.Identity)
                for t in range(4):
                    nc.sync.dma_start(
                        out=logits[t * 128:(t + 1) * 128, voff:voff + vlen],
                        in_=lo[t][:, 0:vlen])

    nc.finalize()
    return nc


def _host_prep(inputs):
    """Build the 8 per-core input maps from the full model inputs."""
    idx = np.asarray(inputs["idx"])
    tok_emb = np.asarray(inputs["tok_emb"], np.float32)
    pos_emb = np.asarray(inputs["pos_emb"], np.float32)
    qkv_w = np.asarray(inputs["qkv_w"], np.float32)
    proj_w = np.asarray(inputs["proj_w"], np.float32)
    fc1_w = np.asarray(inputs["fc1_w"], np.float32)
    fc2_w = np.asarray(inputs["fc2_w"], np.float32)
    head_w = np.asarray(inputs["head_w"], np.float32)
    g1 = np.asarray(inputs["ln1_g"], np.float32)
    b1 = np.asarray(inputs["ln1_b"], np.float32)
    g2 = np.asarray(inputs["ln2_g"], np.float32)
    b2 = np.asarray(inputs["ln2_b"], np.float32)

    # fold LN1 gain into qkv weights, LN2 gain into fc1 weights
    qkv_g = qkv_w * g1[:, None, :]
    fc1_g = fc1_w * g2[:, None, :]
    # LN bias paths
    Bqkv = np.einsum("loe,le->lo", qkv_w, b1)      # [L, 3E]
    bq, bk, bv = Bqkv[:, 0:E], Bqkv[:, E:2 * E], Bqkv[:, 2 * E:3 * E]
    prbias = np.einsum("lef,lf->le", proj_w, bv)   # [L, E]
    fc1_bias = np.asarray(inputs["fc1_b"], np.float32) + \
        np.einsum("lfe,le->lf", fc1_w, b2)

    qkvT = np.ascontiguousarray(qkv_g.transpose(0, 2, 1))    # [L, E, 3E] (q,k,v)
    wqkvT = np.ascontiguousarray(
        np.concatenate([qkvT[:, :, E:2 * E], qkvT[:, :, 2 * E:3 * E],
                        qkvT[:, :, 0:E]], axis=2)).astype(ml_dtypes.bfloat16)
    wprojT = np.ascontiguousarray(
        proj_w.transpose(0, 2, 1)).astype(ml_dtypes.bfloat16)
    w1T = np.ascontiguousarray(
        fc1_g.transpose(0, 2, 1)).astype(ml_dtypes.bfloat16)
    # fc2 lhsT blocks: w2r[l, e, p, uc*128 + m] = fc2_w[l, e*128+m, uc*128+p]
    w2r = np.ascontiguousarray(
        fc2_w.reshape(L, ET, 128, FT, 128).transpose(0, 1, 4, 3, 2)
        .reshape(L, ET, 128, FT * 128)).astype(ml_dtypes.bfloat16)
    headTm = np.ascontiguousarray(head_w.T).astype(ml_dtypes.bfloat16)

    def colsF(v, n):  # [L, n*128] -> [L, 128, n]
        return np.ascontiguousarray(
            np.asarray(v, np.float32).reshape(L, n, 128).transpose(0, 2, 1))

    b1c = colsF(fc1_bias, FT)
    b2c = colsF(np.asarray(inputs["fc2_b"], np.float32), ET)
    prc = colsF(prbias, ET)
    kqc = np.ascontiguousarray(
        np.concatenate([colsF(bk, ET), colsF(bq, ET)], axis=2))  # [L,128,16]
    lnfc = np.ascontiguousarray(np.stack([
        np.asarray(inputs["lnf_g"], np.float32).reshape(ET, 128).T,
        np.asarray(inputs["lnf_b"], np.float32).reshape(ET, 128).T], axis=0))

    # causal mask tiles: M_j[p, f] = (p + 128*j <= f)
    p = np.arange(128)[:, None]
    f = np.arange(TOK)[None, :]
    mj = [(p + 128 * j <= f).astype(np.float32) for j in range(4)]
    zero = np.zeros((128, TOK), np.float32)
    one = np.ones((128, TOK), np.float32)
    m_half0 = np.stack(mj + [zero] * 4).astype(ml_dtypes.bfloat16)
    m_half1 = np.stack([one] * 4 + mj).astype(ml_dtypes.bfloat16)

    x0 = tok_emb[idx] + pos_emb[None, :, :]  # [B, T, E]

    shared = dict(wqkvT=wqkvT, wprojT=wprojT, w1T=w1T, w2r=w2r, b1c=b1c,
                  b2c=b2c, prb=prc, kqb=kqc, lnf=lnfc, headT=headTm,
                  ones_p=np.ones((128, 16), np.float32),
                  ones_bf=np.ones((128, 128), ml_dtypes.bfloat16))
    in_maps = []
    for c in range(NCORES):
        b, half = c // 2, c % 2
        m = dict(shared)
        m["x0T"] = np.ascontiguousarray(
            x0[b, half * TOK:(half + 1) * TOK, :].T).astype(np.float32)
        m["masks"] = np.ascontiguousarray(m_half0 if half == 0 else m_half1)
        in_maps.append(m)
    return in_maps


LAST_EXEC_NS = None


LAST_RES = None


def kernel(trace=False, trace_cores=None, tmpdir=None, **inputs) -> np.ndarray:
    global LAST_EXEC_NS, LAST_RES
    if "nc" not in _CACHED:
        _CACHED["nc"] = _build_nc()
    nc = _CACHED["nc"]
    in_maps = _host_prep(inputs)
    res = run_bass_kernel_spmd(nc, in_maps, core_ids=list(range(NCORES)),
                               trace=trace, trace_cores=trace_cores,
                               tmpdir=tmpdir)
    LAST_RES = res
    LAST_EXEC_NS = res.exec_time_ns
    out = np.empty((B, T, V), np.float32)
    for c in range(NCORES):
        b, half = c // 2, c % 2
        out[b, half * TOK:(half + 1) * TOK, :] = res.results[c]["logits"]
    return out
